# revision 8
# baseline (speedup 1.0000x reference)
"""Trainium2 Bass kernel for nn_Attention_28930899706081 (sparse_attention).

Reference computation:
  k1 = l2norm_c(Wqk @ fmap1), k2 = l2norm_c(Wqk @ fmap2), q = l2norm_c(Wqk @ dmap)
  sim_i = q^T k_i per batch  -> [b, n, n] with n = h*w = 4096
  attn_i = softmax(sim_i, axis=-1)[:, None]  -> [b, 1, n, n]
  returns (attn1, attn2)

Sharding: 8 cores; core i handles batch b = i//4 and query-row block r = i%4
(1024 of 4096 rows). Each core computes the full normalized K for its batch
(recompute instead of collectives) and its row block of both sims + softmax.

Compute dtype bf16 (fp32 accumulation in PSUM); |sim| <= 1 because q/k are
unit vectors, so softmax needs no max subtraction. Row sums come from the
ScalarE activation accumulator fused with exp. Column L2 norms are computed
with a ones-matmul partition reduction (broadcast across partitions), and
1/sqrt comes from the single-pass Abs_reciprocal_sqrt activation (measured
~4e-5 rel err on HW). Output is written bf16 and upcast on the host.
"""

import numpy as np
import ml_dtypes

B, C, H, W, D = 2, 256, 64, 64, 128
N = H * W  # 4096
QBLK = N // 4  # 1024 query rows per core
N_CORES = 8

_cached = {}


def _build():
    import concourse.mybir as mybir
    import concourse.tile as tile
    from concourse.tile_rust import add_dep_helper
    from concourse import bacc
    from contextlib import ExitStack

    f32 = mybir.dt.float32
    bf16 = mybir.dt.bfloat16
    AF = mybir.ActivationFunctionType

    nc = bacc.Bacc(
        "TRN2",
        target_bir_lowering=False,
        debug=False,
        enable_asserts=True,
        num_devices=N_CORES,
    )

    f1_ext = nc.dram_tensor("f1", [C, N], bf16, kind="ExternalInput").ap()
    f2_ext = nc.dram_tensor("f2", [C, N], bf16, kind="ExternalInput").ap()
    xq_ext = nc.dram_tensor("xq", [C, QBLK], bf16, kind="ExternalInput").ap()
    wqkT_ext = nc.dram_tensor("wqkT", [C, D], bf16, kind="ExternalInput").ap()
    out_ext = nc.dram_tensor("out", [2, QBLK, N], bf16, kind="ExternalOutput").ap()

    PCH = 512  # matmul free-dim chunk (one PSUM bank)
    CH = 2048  # pipeline chunk

    with tile.TileContext(nc) as tc, ExitStack() as ctx:
        consts = ctx.enter_context(tc.tile_pool(name="consts", bufs=1))
        xin = ctx.enter_context(tc.tile_pool(name="xin", bufs=8))
        ybf_pool = ctx.enter_context(tc.tile_pool(name="ybf", bufs=3))
        ysq_pool = ctx.enter_context(tc.tile_pool(name="ysq", bufs=4))
        rk_pool = ctx.enter_context(tc.tile_pool(name="rk", bufs=3))
        kn_pool = ctx.enter_context(tc.tile_pool(name="kn", bufs=1))
        e_pool = ctx.enter_context(tc.tile_pool(name="epool", bufs=4))
        attn_pool = ctx.enter_context(tc.tile_pool(name="attn", bufs=2))
        stat_pool = ctx.enter_context(tc.tile_pool(name="stat", bufs=4))

        # constants
        wqkT_sb = [
            consts.tile([128, D], bf16, tag=f"wqkT{k}", name=f"wqkT{k}")
            for k in range(2)
        ]
        nc.sync.dma_start(out=wqkT_sb[0][:], in_=wqkT_ext[0:128, :])
        nc.sync.dma_start(out=wqkT_sb[1][:], in_=wqkT_ext[128:256, :])
        ones_sb = consts.tile([128, 128], bf16, tag="ones", name="ones")
        nc.vector.memset(ones_sb[:], 1.0)

        last_rk = None

        with tc.tile_pool(name="proj_psum", bufs=2, space="PSUM") as proj_psum, \
             tc.tile_pool(name="n2_psum", bufs=1, space="PSUM") as n2_psum:

            def phase_a(x_ext, ncols, tagbase):
                """DMA + project + l2-normalize columns, chunk-pipelined."""
                nonlocal last_rk
                y_bf = ybf_pool.tile([128, N], bf16, tag="ybf", name="y_bf")[:, :ncols]
                xn = kn_pool.tile([128, ncols], bf16, tag=tagbase, name=tagbase)
                XCH = 1024
                ysq_chunks = []
                for h in range(ncols // XCH):
                    h0 = h * XCH
                    x_lo = xin.tile([128, XCH], bf16, tag="xin", name="x_lo")
                    x_hi = xin.tile([128, XCH], bf16, tag="xin", name="x_hi")
                    nc.sync.dma_start(out=x_lo[:], in_=x_ext[0:128, h0 : h0 + XCH])
                    nc.sync.dma_start(out=x_hi[:], in_=x_ext[128:256, h0 : h0 + XCH])

                    ps = proj_psum.tile([128, XCH], f32, tag="proj", name="pps")
                    for c in range(XCH // PCH):
                        sl = slice(c * PCH, (c + 1) * PCH)
                        psl = ps[:, sl]
                        nc.tensor.matmul(
                            psl, wqkT_sb[0][:], x_lo[:, sl], start=True, stop=False
                        )
                        nc.tensor.matmul(
                            psl, wqkT_sb[1][:], x_hi[:, sl], start=False, stop=True
                        )
                    nc.any.tensor_copy(y_bf[:, h0 : h0 + XCH], ps[:])

                    ysq = ysq_pool.tile([128, XCH], bf16, tag="ysq", name="ysq")
                    nc.vector.tensor_mul(ysq[:], y_bf[:, h0 : h0 + XCH], y_bf[:, h0 : h0 + XCH])
                    ysq_chunks.append(ysq)

                for j in range(max(1, ncols // CH)):
                    cw = min(CH, ncols)
                    j0 = j * CH
                    nps = n2_psum.tile([128, CH], f32, tag="n2", name="nps")[:, :cw]
                    for c in range(cw // PCH):
                        ysq = ysq_chunks[(j0 + c * PCH) // XCH]
                        nc.tensor.matmul(
                            nps[:, c * PCH : (c + 1) * PCH],
                            ones_sb[:],
                            ysq[:, (c * PCH) % XCH : (c * PCH) % XCH + PCH],
                            start=True,
                            stop=True,
                        )
                    # rk = n2^-0.5, already broadcast across partitions
                    rk = rk_pool.tile([128, CH], f32, tag="rk", name="rk")[:, :cw]
                    last_rk = nc.scalar.activation(
                        out=rk, in_=nps, func=AF.Abs_reciprocal_sqrt
                    )
                    nc.gpsimd.tensor_mul(
                        xn[:, j0 : j0 + cw], y_bf[:, j0 : j0 + cw], rk
                    )
                return xn

            qn = phase_a(xq_ext, QBLK, "qn")
            k1n = phase_a(f1_ext, N, "k1n")
            k2n = phase_a(f2_ext, N, "k2n")

        with tc.tile_pool(name="sim_psum", bufs=2, space="PSUM") as sim_psum:
            first_exp = None

            def phase_b(kn, s):
                """row block of sim + softmax for one K map, streamed to out[s]."""
                nonlocal first_exp
                for t in range(QBLK // 128):
                    lhsT = qn[:, t * 128 : (t + 1) * 128]
                    attn = attn_pool.tile([128, N], bf16, tag="attn", name="attn")
                    stile = stat_pool.tile([128, 2], f32, tag="stile", name="stile")
                    e_chunks = []
                    for j in range(N // CH):
                        ps = sim_psum.tile([128, CH], f32, tag="sim", name="sim_ps")
                        for c in range(CH // PCH):
                            csl = slice(j * CH + c * PCH, j * CH + (c + 1) * PCH)
                            nc.tensor.matmul(
                                ps[:, c * PCH : (c + 1) * PCH],
                                lhsT,
                                kn[:, csl],
                                start=True,
                                stop=True,
                            )
                        e = e_pool.tile([128, CH], bf16, tag="e", name="e")
                        ex = nc.scalar.activation(
                            out=e[:],
                            in_=ps[:],
                            func=AF.Exp,
                            accum_out=stile[:, j : j + 1],
                        )
                        if first_exp is None:
                            first_exp = ex
                            # keep ACT table loads to 2: all Abs_reciprocal_sqrt
                            # (phase A) strictly before any Exp (phase B)
                            add_dep_helper(
                                ex.ins, last_rk.ins, sync=False,
                                reason="order rk (ars table) before exp table load",
                            )
                        e_chunks.append(e)
                    ssum = stat_pool.tile([128, 1], f32, tag="ssum", name="ssum")
                    nc.vector.reduce_sum(ssum[:], stile[:], axis=mybir.AxisListType.X)
                    recip = stat_pool.tile([128, 1], f32, tag="recip", name="recip")
                    nc.vector.reciprocal(recip[:], ssum[:])
                    for j, e in enumerate(e_chunks):
                        nc.vector.tensor_scalar_mul(
                            attn[:, j * CH : (j + 1) * CH], e[:], recip[:]
                        )
                    nc.sync.dma_start(
                        out=out_ext[s, t * 128 : (t + 1) * 128, :], in_=attn[:]
                    )

            phase_b(k1n, 0)
            phase_b(k2n, 1)

    nc.compile()
    return nc


def _get_nc():
    if "nc" not in _cached:
        _cached["nc"] = _build()
    return _cached["nc"]


def _in_maps(fmap1, fmap2, dmap, Wqk):
    bf = ml_dtypes.bfloat16
    f1r = np.asarray(fmap1, dtype=np.float32).reshape(B, C, N)
    f2r = np.asarray(fmap2, dtype=np.float32).reshape(B, C, N)
    dqr = np.asarray(dmap, dtype=np.float32).reshape(B, C, N)
    wT = np.ascontiguousarray(np.asarray(Wqk, dtype=np.float32).T).astype(bf)

    in_maps = []
    for i in range(N_CORES):
        b, r = divmod(i, 4)
        in_maps.append(
            {
                "f1": np.ascontiguousarray(f1r[b]).astype(bf),
                "f2": np.ascontiguousarray(f2r[b]).astype(bf),
                "xq": np.ascontiguousarray(
                    dqr[b][:, r * QBLK : (r + 1) * QBLK]
                ).astype(bf),
                "wqkT": wT,
            }
        )
    return in_maps


def kernel(fmap1, fmap2, dmap, Wqk):
    from concourse.bass_utils import run_bass_kernel_spmd

    in_maps = _in_maps(fmap1, fmap2, dmap, Wqk)
    nc = _get_nc()
    res = run_bass_kernel_spmd(nc, in_maps, core_ids=list(range(N_CORES)))
    _cached["last_result"] = res

    attn1 = np.empty((B, 1, N, N), dtype=np.float32)
    attn2 = np.empty((B, 1, N, N), dtype=np.float32)
    for i in range(N_CORES):
        b, r = divmod(i, 4)
        o = res.results[i]["out"]
        attn1[b, 0, r * QBLK : (r + 1) * QBLK, :] = o[0].astype(np.float32)
        attn2[b, 0, r * QBLK : (r + 1) * QBLK, :] = o[1].astype(np.float32)
    return (attn1, attn2)


# revision 11
# speedup vs baseline: 1.0110x; 1.0110x over previous
"""Trainium2 Bass kernel for nn_Attention_28930899706081 (sparse_attention).

Reference computation:
  k1 = l2norm_c(Wqk @ fmap1), k2 = l2norm_c(Wqk @ fmap2), q = l2norm_c(Wqk @ dmap)
  sim_i = q^T k_i per batch  -> [b, n, n] with n = h*w = 4096
  attn_i = softmax(sim_i, axis=-1)[:, None]  -> [b, 1, n, n]
  returns (attn1, attn2)

Sharding: 8 cores; core i handles batch b = i//4 and query-row block r = i%4
(1024 of 4096 rows). Each core computes the full normalized K for its batch
(recompute instead of collectives) and its row block of both sims + softmax.

Compute dtype bf16 (fp32 accumulation in PSUM); |sim| <= 1 because q/k are
unit vectors, so softmax needs no max subtraction. Row sums come from the
ScalarE activation accumulator fused with exp. Column L2 norms are computed
with a ones-matmul partition reduction (broadcast across partitions), and
1/sqrt comes from the single-pass Abs_reciprocal_sqrt activation (measured
~4e-5 rel err on HW). Output is written bf16 and upcast on the host.
"""

import numpy as np
import ml_dtypes

B, C, H, W, D = 2, 256, 64, 64, 128
N = H * W  # 4096
QBLK = N // 4  # 1024 query rows per core
N_CORES = 8

_cached = {}


def _build():
    import concourse.mybir as mybir
    import concourse.tile as tile
    from concourse.tile_rust import add_dep_helper
    from concourse import bacc
    from contextlib import ExitStack

    f32 = mybir.dt.float32
    bf16 = mybir.dt.bfloat16
    AF = mybir.ActivationFunctionType

    nc = bacc.Bacc(
        "TRN2",
        target_bir_lowering=False,
        debug=False,
        enable_asserts=True,
        num_devices=N_CORES,
    )

    f1_ext = nc.dram_tensor("f1", [C, N], bf16, kind="ExternalInput").ap()
    f2_ext = nc.dram_tensor("f2", [C, N], bf16, kind="ExternalInput").ap()
    xq_ext = nc.dram_tensor("xq", [C, QBLK], bf16, kind="ExternalInput").ap()
    wqkT_ext = nc.dram_tensor("wqkT", [C, D], bf16, kind="ExternalInput").ap()
    out_ext = nc.dram_tensor("out", [2, QBLK, N], bf16, kind="ExternalOutput").ap()

    PCH = 512  # matmul free-dim chunk (one PSUM bank)
    CH = 2048  # pipeline chunk

    with tile.TileContext(nc) as tc, ExitStack() as ctx:
        consts = ctx.enter_context(tc.tile_pool(name="consts", bufs=1))
        xin = ctx.enter_context(tc.tile_pool(name="xin", bufs=10))
        ysq_pool = ctx.enter_context(tc.tile_pool(name="ysq", bufs=3))
        rk_pool = ctx.enter_context(tc.tile_pool(name="rk", bufs=3))
        kn_pool = ctx.enter_context(tc.tile_pool(name="kn", bufs=1))
        e_pool = ctx.enter_context(tc.tile_pool(name="epool", bufs=6))
        attn_pool = ctx.enter_context(tc.tile_pool(name="attn", bufs=3))
        stat_pool = ctx.enter_context(tc.tile_pool(name="stat", bufs=4))

        # constants
        wqkT_sb = [
            consts.tile([128, D], bf16, tag=f"wqkT{k}", name=f"wqkT{k}")
            for k in range(2)
        ]
        nc.sync.dma_start(out=wqkT_sb[0][:], in_=wqkT_ext[0:128, :])
        nc.sync.dma_start(out=wqkT_sb[1][:], in_=wqkT_ext[128:256, :])
        ones_sb = consts.tile([128, 128], bf16, tag="ones", name="ones")
        nc.vector.memset(ones_sb[:], 1.0)

        last_rk = None

        with tc.tile_pool(name="proj_psum", bufs=3, space="PSUM") as proj_psum, \
             tc.tile_pool(name="n2_psum", bufs=1, space="PSUM") as n2_psum:

            def phase_a(x_ext, ncols, tagbase):
                """DMA + project + l2-normalize columns, chunk-pipelined.

                y = Wqk @ x stays resident in PSUM for the whole chunk chain:
                square (DVE) -> partition-sum (PE ones-matmul) -> 1/sqrt (ACT)
                -> scale y*rk out of PSUM (DVE) -> normalized bf16 in SBUF.
                """
                nonlocal last_rk
                xn = kn_pool.tile([128, ncols], bf16, tag=tagbase, name=tagbase)
                XCH = 1024
                for h in range(ncols // XCH):
                    h0 = h * XCH
                    x_lo = xin.tile([128, XCH], bf16, tag="xin", name="x_lo")
                    x_hi = xin.tile([128, XCH], bf16, tag="xin", name="x_hi")
                    nc.sync.dma_start(out=x_lo[:], in_=x_ext[0:128, h0 : h0 + XCH])
                    nc.sync.dma_start(out=x_hi[:], in_=x_ext[128:256, h0 : h0 + XCH])

                    ps = proj_psum.tile([128, XCH], f32, tag="proj", name="pps")
                    for c in range(XCH // PCH):
                        sl = slice(c * PCH, (c + 1) * PCH)
                        psl = ps[:, sl]
                        nc.tensor.matmul(
                            psl, wqkT_sb[0][:], x_lo[:, sl], start=True, stop=False
                        )
                        nc.tensor.matmul(
                            psl, wqkT_sb[1][:], x_hi[:, sl], start=False, stop=True
                        )

                    ysq = ysq_pool.tile([128, XCH], bf16, tag="ysq", name="ysq")
                    nc.scalar.activation(out=ysq[:], in_=ps[:], func=AF.Square)

                    nps = n2_psum.tile([128, XCH], f32, tag="n2", name="nps")
                    for c in range(XCH // PCH):
                        nc.tensor.matmul(
                            nps[:, c * PCH : (c + 1) * PCH],
                            ones_sb[:],
                            ysq[:, c * PCH : (c + 1) * PCH],
                            start=True,
                            stop=True,
                        )
                    # rk = n2^-0.5, already broadcast across partitions
                    rk = rk_pool.tile([128, XCH], f32, tag="rk", name="rk")
                    last_rk = nc.scalar.activation(
                        out=rk[:], in_=nps[:], func=AF.Abs_reciprocal_sqrt
                    )
                    nc.vector.tensor_mul(xn[:, h0 : h0 + XCH], ps[:], rk[:])
                return xn

            qn = phase_a(xq_ext, QBLK, "qn")
            k1n = phase_a(f1_ext, N, "k1n")
            k2n = phase_a(f2_ext, N, "k2n")

        with tc.tile_pool(name="sim_psum", bufs=2, space="PSUM") as sim_psum:
            first_exp = None

            def phase_b(kn, s):
                """row block of sim + softmax for one K map, streamed to out[s]."""
                nonlocal first_exp
                for t in range(QBLK // 128):
                    lhsT = qn[:, t * 128 : (t + 1) * 128]
                    attn = attn_pool.tile([128, N], bf16, tag="attn", name="attn")
                    stile = stat_pool.tile([128, 2], f32, tag="stile", name="stile")
                    e_chunks = []
                    for j in range(N // CH):
                        ps = sim_psum.tile([128, CH], f32, tag="sim", name="sim_ps")
                        for c in range(CH // PCH):
                            csl = slice(j * CH + c * PCH, j * CH + (c + 1) * PCH)
                            nc.tensor.matmul(
                                ps[:, c * PCH : (c + 1) * PCH],
                                lhsT,
                                kn[:, csl],
                                start=True,
                                stop=True,
                            )
                        e = e_pool.tile([128, CH], bf16, tag="e", name="e")
                        ex = nc.scalar.activation(
                            out=e[:],
                            in_=ps[:],
                            func=AF.Exp,
                            accum_out=stile[:, j : j + 1],
                        )
                        if first_exp is None:
                            first_exp = ex
                            # keep ACT table loads to 2: all Abs_reciprocal_sqrt
                            # (phase A) strictly before any Exp (phase B)
                            add_dep_helper(
                                ex.ins, last_rk.ins, sync=False,
                                reason="order rk (ars table) before exp table load",
                            )
                        e_chunks.append(e)
                    ssum = stat_pool.tile([128, 1], f32, tag="ssum", name="ssum")
                    nc.vector.reduce_sum(ssum[:], stile[:], axis=mybir.AxisListType.X)
                    recip = stat_pool.tile([128, 1], f32, tag="recip", name="recip")
                    nc.vector.reciprocal(recip[:], ssum[:])
                    for j, e in enumerate(e_chunks):
                        nc.vector.tensor_scalar_mul(
                            attn[:, j * CH : (j + 1) * CH], e[:], recip[:]
                        )
                    nc.sync.dma_start(
                        out=out_ext[s, t * 128 : (t + 1) * 128, :], in_=attn[:]
                    )

            phase_b(k1n, 0)
            phase_b(k2n, 1)

    nc.compile()
    return nc


def _get_nc():
    if "nc" not in _cached:
        _cached["nc"] = _build()
    return _cached["nc"]


def _in_maps(fmap1, fmap2, dmap, Wqk):
    bf = ml_dtypes.bfloat16
    f1r = np.asarray(fmap1, dtype=np.float32).reshape(B, C, N)
    f2r = np.asarray(fmap2, dtype=np.float32).reshape(B, C, N)
    dqr = np.asarray(dmap, dtype=np.float32).reshape(B, C, N)
    wT = np.ascontiguousarray(np.asarray(Wqk, dtype=np.float32).T).astype(bf)

    in_maps = []
    for i in range(N_CORES):
        b, r = divmod(i, 4)
        in_maps.append(
            {
                "f1": np.ascontiguousarray(f1r[b]).astype(bf),
                "f2": np.ascontiguousarray(f2r[b]).astype(bf),
                "xq": np.ascontiguousarray(
                    dqr[b][:, r * QBLK : (r + 1) * QBLK]
                ).astype(bf),
                "wqkT": wT,
            }
        )
    return in_maps


def kernel(fmap1, fmap2, dmap, Wqk):
    from concourse.bass_utils import run_bass_kernel_spmd

    in_maps = _in_maps(fmap1, fmap2, dmap, Wqk)
    nc = _get_nc()
    res = run_bass_kernel_spmd(nc, in_maps, core_ids=list(range(N_CORES)))
    _cached["last_result"] = res

    attn1 = np.empty((B, 1, N, N), dtype=np.float32)
    attn2 = np.empty((B, 1, N, N), dtype=np.float32)
    for i in range(N_CORES):
        b, r = divmod(i, 4)
        o = res.results[i]["out"]
        attn1[b, 0, r * QBLK : (r + 1) * QBLK, :] = o[0].astype(np.float32)
        attn2[b, 0, r * QBLK : (r + 1) * QBLK, :] = o[1].astype(np.float32)
    return (attn1, attn2)
